# revision 1
# baseline (speedup 1.0000x reference)
"""CrossCoder kernel for 8 Trainium2 NeuronCores (Bass/Tile, SPMD).

Math (reference):
    f     = relu(einsum('bld,ldf->bf', x, W_enc) + b_enc)     # [B, F]
    x_hat = einsum('bf,lfd->bld', f, W_dec) + b_dec           # [B, L, D]

Sharding: dict dim F=32768 split 8 ways (FL=4096 per core, tensor parallel
over latents). Each core computes its local f shard (encode) and the
partial decode sum over its latents; ReduceScatters combine the partials,
leaving each core with a distinct slice of the (LD=2048, B) transposed
output, which the host reassembles and transposes back.

Device layout is feature-major (contraction dim on SBUF partitions); batch
runs in two halves of 512 inside ONE TileContext. Collectives are emitted
in-context: RS0 after half 0 overlaps all of half 1; half 1's partial is
split in two (ld rows 0-1023 / 1024-2047) so RS1a overlaps the tail of the
decode and only RS1b (2MB) is exposed. Weights/x are host-repacked into
contiguous [128, 512] tiles so every DMA is one 256KB contiguous block.
b_dec/8 is folded in pre-collective. All matmuls are float32r (full PE
rate, ~2e-4 rel err).
"""

import numpy as np

B = 1024
L = 2
D = 1024
F = 32768
NCORES = 8
FL = F // NCORES      # 4096 latents per core
LD = L * D            # 2048
KT = LD // 128        # 16 encode k-tiles
FT = FL // 128        # 32 f-tiles per core
NB = 512              # matmul moving free dim
NH = 2                # batch halves

_CACHE = {}


def _build_nc():
    import concourse.bass as bass  # noqa: F401
    import concourse.tile as tile
    from concourse import bacc, mybir

    f32 = mybir.dt.float32
    f32r = mybir.dt.float32r

    nc = bacc.Bacc()

    xT = nc.declare_dram_parameter("xT", [NH, KT, 128, NB], f32r, isOutput=False)
    w_enc = nc.declare_dram_parameter("w_enc", [KT, FT // 4, 128, NB], f32r, isOutput=False)
    w_dec = nc.declare_dram_parameter("w_dec", [L, 2, FT, 128, NB], f32r, isOutput=False)
    b_enc = nc.declare_dram_parameter("b_enc", [128, FT], f32, isOutput=False)
    b_dec8 = nc.declare_dram_parameter("b_dec8", [128, KT], f32, isOutput=False)
    # out_sh: [0:2] = h0 ld-tiles {2i,2i+1}; [2] = h1 ld-tile i; [3] = h1 ld-tile 8+i
    out_sh = nc.declare_dram_parameter("out_sh", [4, 128, NB], f32, isOutput=True)

    # partial buffers: one per (half, l-block) so each ReduceScatter fires as
    # soon as its 8 ld-tiles are written, spreading collective traffic
    partial0 = nc.dram_tensor("partial0", [KT, 128, NB], f32)
    parts1 = [nc.dram_tensor(f"partial1{l}", [KT // 2, 128, NB], f32) for l in range(L)]
    rs0 = nc.dram_tensor("rs0", [2, 128, NB], f32)
    rss1 = [nc.dram_tensor(f"rs1{l}", [1, 128, NB], f32) for l in range(L)]

    xT_a = xT.ap()
    w_enc_a = w_enc.ap()
    w_dec_a = w_dec.ap()
    rgroups = [list(range(NCORES))]

    with tile.TileContext(nc) as tc:
        with (
            tc.tile_pool(name="xp", bufs=1) as xp,
            tc.tile_pool(name="fp", bufs=1) as fp,
            tc.tile_pool(name="we", bufs=16) as we,
            tc.tile_pool(name="wd", bufs=16) as wd,
            tc.tile_pool(name="stg", bufs=8) as stg,
            tc.tile_pool(name="bias", bufs=1) as bias,
            tc.tile_pool(name="ps", bufs=8, space="PSUM") as ps,
        ):
            benc_t = bias.tile([128, FT], f32, name="benc")
            nc.sync.dma_start(out=benc_t, in_=b_enc.ap())
            bdec_t = bias.tile([128, KT], f32, name="bdec")
            nc.sync.dma_start(out=bdec_t, in_=b_dec8.ap())

            for h in range(NH):
                x_tiles = []
                for k in range(KT):
                    xt = xp.tile([128, NB], f32r, tag=f"x{k}", name=f"x{k}")
                    nc.sync.dma_start(out=xt, in_=xT_a[h, k])
                    x_tiles.append(xt)

                # ---- encode
                f_tiles = []
                for fg in range(FT // 4):
                    pss = [
                        ps.tile([128, NB], f32, tag="ps", name=f"pse{_j}")
                        for _j in range(4)
                    ]
                    for k in range(KT):
                        wt = we.tile([128, NB], f32r, tag="we", name="wet")
                        nc.sync.dma_start(out=wt, in_=w_enc_a[k, fg])
                        for j in range(4):
                            nc.tensor.matmul(
                                pss[j],
                                wt[:, j * 128 : (j + 1) * 128],
                                x_tiles[k],
                                start=(k == 0),
                                stop=(k == KT - 1),
                            )
                    for j in range(4):
                        ft_idx = fg * 4 + j
                        ftile = fp.tile(
                            [128, NB], f32r, tag=f"f{ft_idx}", name=f"f{ft_idx}"
                        )
                        nc.scalar.activation(
                            ftile,
                            pss[j],
                            mybir.ActivationFunctionType.Relu,
                            bias=benc_t[:, ft_idx : ft_idx + 1],
                        )
                        f_tiles.append(ftile)

                # ---- decode
                for l in range(L):
                    part_a = partial0.ap() if h == 0 else parts1[l].ap()
                    base = l * 8 if h == 0 else 0
                    for dg in range(2):
                        pss = [
                            ps.tile([128, NB], f32, tag="ps", name=f"psd{_j}")
                            for _j in range(4)
                        ]
                        for fk in range(FT):
                            wt = wd.tile([128, NB], f32r, tag="wd", name="wdt")
                            nc.sync.dma_start(out=wt, in_=w_dec_a[l, dg, fk])
                            for j in range(4):
                                nc.tensor.matmul(
                                    pss[j],
                                    wt[:, j * 128 : (j + 1) * 128],
                                    f_tiles[fk],
                                    start=(fk == 0),
                                    stop=(fk == FT - 1),
                                )
                        for j in range(4):
                            ld_t = l * 8 + dg * 4 + j
                            st = stg.tile([128, NB], f32, tag="st", name="st")
                            nc.vector.tensor_scalar_add(
                                st, pss[j], bdec_t[:, ld_t : ld_t + 1]
                            )
                            nc.sync.dma_start(
                                out=part_a[base + dg * 4 + j], in_=st
                            )
                    if h == 1:
                        # this l-block's partial is complete → ReduceScatter it
                        nc.gpsimd.collective_compute(
                            "ReduceScatter",
                            mybir.AluOpType.add,
                            ins=[parts1[l][:]],
                            outs=[rss1[l][:]],
                            replica_groups=rgroups,
                        )
                if h == 0:
                    nc.gpsimd.collective_compute(
                        "ReduceScatter",
                        mybir.AluOpType.add,
                        ins=[partial0[:]],
                        outs=[rs0[:]],
                        replica_groups=rgroups,
                    )

            out_a = out_sh.ap()
            nc.gpsimd.dma_start(out=out_a[0:2], in_=rs0[:])
            nc.gpsimd.dma_start(out=out_a[2:3], in_=rss1[0][:])
            nc.gpsimd.dma_start(out=out_a[3:4], in_=rss1[1][:])

    nc.finalize()
    return nc


def _get_nc():
    if "nc" not in _CACHE:
        _CACHE["nc"] = _build_nc()
    return _CACHE["nc"]


def kernel(x, W_enc, b_enc, W_dec, b_dec):
    from concourse.bass_utils import run_bass_kernel_spmd

    x = np.asarray(x, dtype=np.float32)
    W_enc = np.asarray(W_enc, dtype=np.float32)
    b_enc = np.asarray(b_enc, dtype=np.float32)
    W_dec = np.asarray(W_dec, dtype=np.float32)
    b_dec = np.asarray(b_dec, dtype=np.float32)

    nc = _get_nc()

    # xT blocked: [h, k, p, c] with xT row k*128+p (= x.reshape(B,LD).T), col h*512+c
    xT = np.ascontiguousarray(
        x.reshape(B, LD).T.reshape(KT, 128, NH, NB).transpose(2, 0, 1, 3)
    )
    w_enc_flat = W_enc.reshape(LD, F)
    bdec8 = np.ascontiguousarray(
        (b_dec.reshape(LD) / NCORES).astype(np.float32).reshape(KT, 128).T
    )

    in_maps = []
    for i in range(NCORES):
        fsl = slice(i * FL, (i + 1) * FL)
        we_blk = np.ascontiguousarray(
            w_enc_flat[:, fsl].reshape(KT, 128, FT // 4, NB).transpose(0, 2, 1, 3)
        )
        wd_blk = np.ascontiguousarray(
            W_dec[:, fsl, :].reshape(L, FT, 128, 2, NB).transpose(0, 3, 1, 2, 4)
        )
        in_maps.append(
            {
                "xT": xT,
                "w_enc": we_blk,
                "w_dec": wd_blk,
                "b_enc": np.ascontiguousarray(b_enc[fsl].reshape(FT, 128).T),
                "b_dec8": bdec8,
            }
        )

    res = run_bass_kernel_spmd(nc, in_maps, list(range(NCORES)))
    _CACHE["last_res"] = res

    xhatT = np.empty((LD, B), dtype=np.float32)
    for i in range(NCORES):
        arr = res.results[i]["out_sh"]  # [4, 128, NB]
        xhatT[2 * i * 128 : (2 * i + 2) * 128, 0:NB] = arr[0:2].reshape(256, NB)
        xhatT[i * 128 : (i + 1) * 128, NB : 2 * NB] = arr[2]
        xhatT[(8 + i) * 128 : (9 + i) * 128, NB : 2 * NB] = arr[3]
    return np.ascontiguousarray(xhatT.T).reshape(B, L, D).astype(np.float32)



# revision 2
# speedup vs baseline: 1.1334x; 1.1334x over previous
"""CrossCoder kernel for 8 Trainium2 NeuronCores (Bass/Tile, SPMD).

Math (reference):
    f     = relu(einsum('bld,ldf->bf', x, W_enc) + b_enc)     # [B, F]
    x_hat = einsum('bf,lfd->bld', f, W_dec) + b_dec           # [B, L, D]

Sharding: dict dim F=32768 split 8 ways (FL=4096 per core, tensor parallel
over latents). Each core computes its local f shard (encode) and the
partial decode sum over its latents; ReduceScatters combine the partials,
leaving each core with a distinct (ld-tile, batch-half) slice of the
transposed output, which the host reassembles.

All matmul operands are bf16 (PSUM accumulation stays fp32): bf16 gets the
fast-weight-load path + LDWEIGHTS hidden behind the moving phase
(~216 ns/MM vs fp32r's ~272), and halves weight DMA. Weights are streamed
ONCE — each [128,512] weight tile feeds 8 matmuls (4 PE-column chunks x 2
batch halves) accumulating into all 8 PSUM banks, so both batch halves are
computed per load. Decode emits 4 partial pieces (one per (l, dg) block of
4 ld-tiles); each 2MB piece is ReduceScattered as soon as its block
finishes, overlapping the next block; only the last piece's RS is exposed.
b_dec/8 is folded in pre-collective. Host repacks weights/x into
contiguous [128,512]/[128,1024] tiles so every DMA is one contiguous block.
"""

import numpy as np
import ml_dtypes

B = 1024
L = 2
D = 1024
F = 32768
NCORES = 8
FL = F // NCORES      # 4096 latents per core
LD = L * D            # 2048
KT = LD // 128        # 16 encode k-tiles
FT = FL // 128        # 32 f-tiles per core
FG = FT // 4          # 8 encode f-groups (512 f-cols each)
NB = 512              # matmul moving free dim (PSUM bank capacity in fp32)
NH = 2                # batch halves

_CACHE = {}


def _build_nc():
    import concourse.bass as bass  # noqa: F401
    import concourse.tile as tile
    from concourse import bacc, mybir

    f32 = mybir.dt.float32
    bf16 = mybir.dt.bfloat16

    nc = bacc.Bacc()

    xT = nc.declare_dram_parameter("xT", [KT, 128, B], bf16, isOutput=False)
    w_enc = nc.declare_dram_parameter("w_enc", [FG, KT, 128, NB], bf16, isOutput=False)
    w_dec = nc.declare_dram_parameter("w_dec", [L, 2, FT, 128, NB], bf16, isOutput=False)
    b_enc = nc.declare_dram_parameter("b_enc", [128, FT], f32, isOutput=False)
    b_dec8 = nc.declare_dram_parameter("b_dec8", [128, KT], f32, isOutput=False)
    # out_sh[p] for p = 2*l + dg: this core's RS shard of decode block (l, dg)
    out_sh = nc.declare_dram_parameter("out_sh", [4, 128, NB], f32, isOutput=True)

    # one partial buffer + RS output per (l, dg) decode block, so each
    # ReduceScatter fires as soon as its block's 8 tiles are written
    parts = [nc.dram_tensor(f"partial{p}", [8, 128, NB], f32) for p in range(4)]
    rss = [nc.dram_tensor(f"rs{p}", [1, 128, NB], f32) for p in range(4)]

    xT_a = xT.ap()
    w_enc_a = w_enc.ap()
    w_dec_a = w_dec.ap()
    out_a = out_sh.ap()
    rgroups = [list(range(NCORES))]

    with tile.TileContext(nc) as tc:
        with (
            tc.tile_pool(name="xp", bufs=1) as xp,
            tc.tile_pool(name="fp", bufs=1) as fp,
            tc.tile_pool(name="we", bufs=10) as we,
            tc.tile_pool(name="wd", bufs=10) as wd,
            tc.tile_pool(name="stg", bufs=8) as stg,
            tc.tile_pool(name="bias", bufs=1) as bias,
            tc.tile_pool(name="ps", bufs=8, space="PSUM") as ps,
        ):
            benc_t = bias.tile([128, FT], f32, name="benc")
            nc.sync.dma_start(out=benc_t, in_=b_enc.ap())
            bdec_t = bias.tile([128, KT], f32, name="bdec")
            nc.sync.dma_start(out=bdec_t, in_=b_dec8.ap())

            x_tiles = []
            for k in range(KT):
                xt = xp.tile([128, B], bf16, tag=f"x{k}", name=f"x{k}")
                nc.sync.dma_start(out=xt, in_=xT_a[k])
                x_tiles.append(xt)

            # ---- encode: f[h] = relu(xT.T @ W_enc + b_enc), both halves
            # per weight tile load
            f_tiles = []  # [fk][h]
            for fg in range(FG):
                pss = [
                    ps.tile([128, NB], f32, tag="ps", name=f"pse{_j}")
                    for _j in range(8)
                ]
                for k in range(KT):
                    wt = we.tile([128, NB], bf16, tag="we", name="wet")
                    nc.sync.dma_start(out=wt, in_=w_enc_a[fg, k])
                    for j in range(4):
                        for h in range(NH):
                            nc.tensor.matmul(
                                pss[j * 2 + h],
                                wt[:, j * 128 : (j + 1) * 128],
                                x_tiles[k][:, h * NB : (h + 1) * NB],
                                start=(k == 0),
                                stop=(k == KT - 1),
                            )
                for j in range(4):
                    ft_idx = fg * 4 + j
                    pair = []
                    for h in range(NH):
                        ftile = fp.tile(
                            [128, NB], bf16, tag=f"f{ft_idx}_{h}", name=f"f{ft_idx}_{h}"
                        )
                        nc.scalar.activation(
                            ftile,
                            pss[j * 2 + h],
                            mybir.ActivationFunctionType.Relu,
                            bias=benc_t[:, ft_idx : ft_idx + 1],
                        )
                        pair.append(ftile)
                    f_tiles.append(pair)

            # ---- decode: partial[ld, b] = W_dec.T @ f, both halves per load
            for l in range(L):
                for dg in range(2):
                    p = l * 2 + dg
                    pss = [
                        ps.tile([128, NB], f32, tag="ps", name=f"psd{_j}")
                        for _j in range(8)
                    ]
                    for fk in range(FT):
                        wt = wd.tile([128, NB], bf16, tag="wd", name="wdt")
                        nc.sync.dma_start(out=wt, in_=w_dec_a[l, dg, fk])
                        for j in range(4):
                            for h in range(NH):
                                nc.tensor.matmul(
                                    pss[j * 2 + h],
                                    wt[:, j * 128 : (j + 1) * 128],
                                    f_tiles[fk][h],
                                    start=(fk == 0),
                                    stop=(fk == FT - 1),
                                )
                    part_a = parts[p].ap()
                    for j in range(4):
                        ld_t = l * 8 + dg * 4 + j
                        for h in range(NH):
                            st = stg.tile([128, NB], f32, tag="st", name="st")
                            nc.vector.tensor_scalar_add(
                                st, pss[j * 2 + h], bdec_t[:, ld_t : ld_t + 1]
                            )
                            nc.sync.dma_start(out=part_a[j * 2 + h], in_=st)
                    # block's partial is complete -> ReduceScatter it; core i
                    # receives tile (j=i//2, h=i%2)
                    nc.gpsimd.collective_compute(
                        "ReduceScatter",
                        mybir.AluOpType.add,
                        ins=[parts[p][:]],
                        outs=[rss[p][:]],
                        replica_groups=rgroups,
                    )
                    nc.gpsimd.dma_start(out=out_a[p : p + 1], in_=rss[p][:])

    nc.finalize()
    return nc


def _get_nc():
    if "nc" not in _CACHE:
        _CACHE["nc"] = _build_nc()
    return _CACHE["nc"]


def kernel(x, W_enc, b_enc, W_dec, b_dec):
    from concourse.bass_utils import run_bass_kernel_spmd

    x = np.asarray(x, dtype=np.float32)
    W_enc = np.asarray(W_enc, dtype=np.float32)
    b_enc = np.asarray(b_enc, dtype=np.float32)
    W_dec = np.asarray(W_dec, dtype=np.float32)
    b_dec = np.asarray(b_dec, dtype=np.float32)

    nc = _get_nc()

    bf = ml_dtypes.bfloat16
    # xT row k*128+p (= x.reshape(B,LD).T), col b
    xT = np.ascontiguousarray(x.reshape(B, LD).T.reshape(KT, 128, B)).astype(bf)
    w_enc_flat = W_enc.reshape(LD, F)
    bdec8 = np.ascontiguousarray(
        (b_dec.reshape(LD) / NCORES).astype(np.float32).reshape(KT, 128).T
    )

    in_maps = []
    for i in range(NCORES):
        fsl = slice(i * FL, (i + 1) * FL)
        we_blk = (
            w_enc_flat[:, fsl].reshape(KT, 128, FG, NB).transpose(2, 0, 1, 3)
        ).astype(bf)
        wd_blk = (
            W_dec[:, fsl, :].reshape(L, FT, 128, 2, NB).transpose(0, 3, 1, 2, 4)
        ).astype(bf)
        in_maps.append(
            {
                "xT": xT,
                "w_enc": np.ascontiguousarray(we_blk),
                "w_dec": np.ascontiguousarray(wd_blk),
                "b_enc": np.ascontiguousarray(b_enc[fsl].reshape(FT, 128).T),
                "b_dec8": bdec8,
            }
        )

    res = run_bass_kernel_spmd(nc, in_maps, list(range(NCORES)))
    _CACHE["last_res"] = res

    xhatT = np.empty((LD, B), dtype=np.float32)
    for i in range(NCORES):
        arr = res.results[i]["out_sh"]  # [4, 128, NB]
        j, h = i // 2, i % 2
        for p in range(4):
            l, dg = p // 2, p % 2
            r0 = 128 * (l * 8 + dg * 4 + j)
            xhatT[r0 : r0 + 128, h * NB : (h + 1) * NB] = arr[p]
    return np.ascontiguousarray(xhatT.T).reshape(B, L, D).astype(np.float32)


# revision 3
# speedup vs baseline: 1.1679x; 1.0304x over previous
"""CrossCoder kernel for 8 Trainium2 NeuronCores (Bass/Tile, SPMD).

Math (reference):
    f     = relu(einsum('bld,ldf->bf', x, W_enc) + b_enc)     # [B, F]
    x_hat = einsum('bf,lfd->bld', f, W_dec) + b_dec           # [B, L, D]

Sharding: dict dim F=32768 split 8 ways (FL=4096 per core, tensor parallel
over latents). Each core computes its local f shard (encode) and the
partial decode sum over its latents; ReduceScatters combine the partials,
leaving each core with a distinct (ld-tile, batch-half) slice of the
transposed output, which the host reassembles.

Perf structure (PE sustains ~263ns per [128x128x512] matmul at the P0
sustained clock; 2048 MMs/core = ~538us is the floor):
- all matmul operands bf16 (FWL + LDWEIGHTS fully hidden), PSUM fp32
- weights streamed ONCE: each [128,512] weight tile feeds 8 matmuls
  (4 PE-column chunks x 2 batch halves) into all 8 PSUM banks
- x tiles DMA'd on the GpSimd queue and biases on the Scalar queue so the
  Sync queue's first transfer is the first weight tile (startup latency)
- 8 dummy warmup matmuls during the initial DMA window release the HAM
  clock throttle before the real stream begins
- decode partials/collectives in bf16; decode block (l,dg) ReduceScatters
  as soon as its 8 tiles land, overlapping the next block; the final block
  is split into two 4-bank sub-blocks (sharing one resident weight load)
  so only a 0.5MB RS piece remains exposed at the end
- evacuations alternate Vector/Scalar engines to halve the drain at block
  boundaries; b_dec/8 is folded in pre-collective
"""

import numpy as np
import ml_dtypes

B = 1024
L = 2
D = 1024
F = 32768
NCORES = 8
FL = F // NCORES      # 4096 latents per core
LD = L * D            # 2048
KT = LD // 128        # 16 encode k-tiles
FT = FL // 128        # 32 f-tiles per core
FG = FT // 4          # 8 encode f-groups (512 f-cols each)
NB = 512              # matmul moving free dim (PSUM bank capacity in fp32)
NH = 2                # batch halves

_CACHE = {}


def _build_nc():
    import concourse.bass as bass  # noqa: F401
    import concourse.tile as tile
    from concourse import bacc, mybir

    f32 = mybir.dt.float32
    bf16 = mybir.dt.bfloat16
    Relu = mybir.ActivationFunctionType.Relu
    Identity = mybir.ActivationFunctionType.Identity

    nc = bacc.Bacc()

    xT = nc.declare_dram_parameter("xT", [KT, 128, B], bf16, isOutput=False)
    w_enc = nc.declare_dram_parameter("w_enc", [FG, KT, 128, NB], bf16, isOutput=False)
    w_dec = nc.declare_dram_parameter("w_dec", [L, 2, FT, 128, NB], bf16, isOutput=False)
    b_enc = nc.declare_dram_parameter("b_enc", [128, FT], f32, isOutput=False)
    b_dec8 = nc.declare_dram_parameter("b_dec8", [128, KT], f32, isOutput=False)
    # out_sh[p], p = 2*l + dg in {0,1,2}: this core's RS shard of block (l,dg)
    # out_sh2[sp]: shards of the two split sub-blocks of (l=1, dg=1)
    out_sh = nc.declare_dram_parameter("out_sh", [3, 128, NB], bf16, isOutput=True)
    out_sh2 = nc.declare_dram_parameter("out_sh2", [2, 64, NB], bf16, isOutput=True)

    parts = [nc.dram_tensor(f"partial{p}", [8, 128, NB], bf16) for p in range(3)]
    rss = [nc.dram_tensor(f"rs{p}", [1, 128, NB], bf16) for p in range(3)]
    parts2 = [nc.dram_tensor(f"partial3{sp}", [4, 128, NB], bf16) for sp in range(2)]
    rss2 = [nc.dram_tensor(f"rs3{sp}", [64, NB], bf16) for sp in range(2)]

    xT_a = xT.ap()
    w_enc_a = w_enc.ap()
    w_dec_a = w_dec.ap()
    out_a = out_sh.ap()
    out2_a = out_sh2.ap()
    rgroups = [list(range(NCORES))]

    with tile.TileContext(nc) as tc:
        with (
            tc.tile_pool(name="xp", bufs=1) as xp,
            tc.tile_pool(name="fp", bufs=1) as fp,
            tc.tile_pool(name="we", bufs=12) as we,
            tc.tile_pool(name="wd", bufs=12) as wd,
            tc.tile_pool(name="wdl", bufs=1) as wdl,
            tc.tile_pool(name="stg", bufs=8) as stg,
            tc.tile_pool(name="bias", bufs=1) as bias,
            tc.tile_pool(name="ps", bufs=8, space="PSUM") as ps,
        ):
            # ---- PE warmup: release the HAM clock throttle with dummy
            # matmuls while the first real DMAs are in flight
            warm_w = bias.tile([128, 128], bf16, name="warmw")
            warm_x = bias.tile([128, NB], bf16, name="warmx")
            nc.vector.memset(warm_w[:], 0.0)
            nc.vector.memset(warm_x[:], 0.0)
            warm_ps = ps.tile([128, NB], f32, tag="ps", name="warmps")
            for i in range(8):
                nc.tensor.matmul(
                    warm_ps, warm_w, warm_x, start=(i == 0), stop=(i == 7)
                )

            # biases on the Scalar DMA queue, x on GpSimd: the Sync queue's
            # first transfer is then w_enc[0,0]
            benc_t = bias.tile([128, FT], f32, name="benc")
            nc.scalar.dma_start(out=benc_t, in_=b_enc.ap())
            bdec_t = bias.tile([128, KT], f32, name="bdec")
            nc.scalar.dma_start(out=bdec_t, in_=b_dec8.ap())

            x_tiles = []
            for k in range(KT):
                xt = xp.tile([128, B], bf16, tag=f"x{k}", name=f"x{k}")
                nc.gpsimd.dma_start(out=xt, in_=xT_a[k])
                x_tiles.append(xt)

            # ---- encode: f[h] = relu(xT.T @ W_enc + b_enc), both halves
            # per weight tile load
            f_tiles = []  # [fk][h]
            for fg in range(FG):
                pss = [
                    ps.tile([128, NB], f32, tag="ps", name=f"pse{_j}")
                    for _j in range(8)
                ]
                for k in range(KT):
                    wt = we.tile([128, NB], bf16, tag="we", name="wet")
                    nc.sync.dma_start(out=wt, in_=w_enc_a[fg, k])
                    for j in range(4):
                        for h in range(NH):
                            nc.tensor.matmul(
                                pss[j * 2 + h],
                                wt[:, j * 128 : (j + 1) * 128],
                                x_tiles[k][:, h * NB : (h + 1) * NB],
                                start=(k == 0),
                                stop=(k == KT - 1),
                            )
                for j in range(4):
                    ft_idx = fg * 4 + j
                    pair = []
                    for h in range(NH):
                        ftile = fp.tile(
                            [128, NB], bf16, tag=f"f{ft_idx}_{h}", name=f"f{ft_idx}_{h}"
                        )
                        nc.scalar.activation(
                            ftile,
                            pss[j * 2 + h],
                            Relu,
                            bias=benc_t[:, ft_idx : ft_idx + 1],
                        )
                        pair.append(ftile)
                    f_tiles.append(pair)

            def evac(pst, ld_t, h, out_ap):
                """PSUM -> (+ b_dec/8) -> bf16 staging -> DRAM partial."""
                st = stg.tile([128, NB], bf16, tag="st", name="st")
                if h == 0:
                    nc.vector.tensor_scalar_add(
                        st, pst, bdec_t[:, ld_t : ld_t + 1]
                    )
                else:
                    nc.scalar.activation(
                        st, pst, Identity, bias=bdec_t[:, ld_t : ld_t + 1]
                    )
                nc.sync.dma_start(out=out_ap, in_=st)

            # ---- decode: partial[ld, b] = W_dec.T @ f, both halves per load
            for l in range(L):
                for dg in range(2):
                    p = l * 2 + dg
                    if p < 3:
                        pss = [
                            ps.tile([128, NB], f32, tag="ps", name=f"psd{_j}")
                            for _j in range(8)
                        ]
                        for fk in range(FT):
                            wt = wd.tile([128, NB], bf16, tag="wd", name="wdt")
                            nc.sync.dma_start(out=wt, in_=w_dec_a[l, dg, fk])
                            for j in range(4):
                                for h in range(NH):
                                    nc.tensor.matmul(
                                        pss[j * 2 + h],
                                        wt[:, j * 128 : (j + 1) * 128],
                                        f_tiles[fk][h],
                                        start=(fk == 0),
                                        stop=(fk == FT - 1),
                                    )
                        part_a = parts[p].ap()
                        for j in range(4):
                            ld_t = l * 8 + dg * 4 + j
                            for h in range(NH):
                                evac(pss[j * 2 + h], ld_t, h, part_a[j * 2 + h])
                        nc.gpsimd.collective_compute(
                            "ReduceScatter",
                            mybir.AluOpType.add,
                            ins=[parts[p][:]],
                            outs=[rss[p][:]],
                            replica_groups=rgroups,
                        )
                        nc.gpsimd.dma_start(out=out_a[p : p + 1], in_=rss[p][:])
                    else:
                        # final block: two 4-bank sub-blocks over a single
                        # resident weight load; each fires its own small RS
                        wl_tiles = []
                        for fk in range(FT):
                            wt = wdl.tile(
                                [128, NB], bf16, tag=f"wl{fk}", name=f"wl{fk}"
                            )
                            nc.sync.dma_start(out=wt, in_=w_dec_a[l, dg, fk])
                            wl_tiles.append(wt)
                        for sp in range(2):
                            pss = [
                                ps.tile([128, NB], f32, tag="ps", name=f"pss{_j}")
                                for _j in range(4)
                            ]
                            for fk in range(FT):
                                for jl in range(2):
                                    j = sp * 2 + jl
                                    for h in range(NH):
                                        nc.tensor.matmul(
                                            pss[jl * 2 + h],
                                            wl_tiles[fk][:, j * 128 : (j + 1) * 128],
                                            f_tiles[fk][h],
                                            start=(fk == 0),
                                            stop=(fk == FT - 1),
                                        )
                            part_a = parts2[sp].ap()
                            for jl in range(2):
                                j = sp * 2 + jl
                                ld_t = l * 8 + dg * 4 + j
                                for h in range(NH):
                                    evac(pss[jl * 2 + h], ld_t, h, part_a[jl * 2 + h])
                            nc.gpsimd.collective_compute(
                                "ReduceScatter",
                                mybir.AluOpType.add,
                                ins=[parts2[sp][:]],
                                outs=[rss2[sp][:]],
                                replica_groups=rgroups,
                            )
                            nc.gpsimd.dma_start(out=out2_a[sp], in_=rss2[sp][:])

    nc.finalize()
    return nc


def _get_nc():
    if "nc" not in _CACHE:
        _CACHE["nc"] = _build_nc()
    return _CACHE["nc"]


def kernel(x, W_enc, b_enc, W_dec, b_dec):
    from concourse.bass_utils import run_bass_kernel_spmd

    x = np.asarray(x, dtype=np.float32)
    W_enc = np.asarray(W_enc, dtype=np.float32)
    b_enc = np.asarray(b_enc, dtype=np.float32)
    W_dec = np.asarray(W_dec, dtype=np.float32)
    b_dec = np.asarray(b_dec, dtype=np.float32)

    nc = _get_nc()

    bf = ml_dtypes.bfloat16
    # xT row k*128+p (= x.reshape(B,LD).T), col b
    xT = np.ascontiguousarray(x.reshape(B, LD).T.reshape(KT, 128, B)).astype(bf)
    w_enc_flat = W_enc.reshape(LD, F)
    bdec8 = np.ascontiguousarray(
        (b_dec.reshape(LD) / NCORES).astype(np.float32).reshape(KT, 128).T
    )

    in_maps = []
    for i in range(NCORES):
        fsl = slice(i * FL, (i + 1) * FL)
        we_blk = (
            w_enc_flat[:, fsl].reshape(KT, 128, FG, NB).transpose(2, 0, 1, 3)
        ).astype(bf)
        wd_blk = (
            W_dec[:, fsl, :].reshape(L, FT, 128, 2, NB).transpose(0, 3, 1, 2, 4)
        ).astype(bf)
        in_maps.append(
            {
                "xT": xT,
                "w_enc": np.ascontiguousarray(we_blk),
                "w_dec": np.ascontiguousarray(wd_blk),
                "b_enc": np.ascontiguousarray(b_enc[fsl].reshape(FT, 128).T),
                "b_dec8": bdec8,
            }
        )

    res = run_bass_kernel_spmd(nc, in_maps, list(range(NCORES)))
    _CACHE["last_res"] = res

    xhatT = np.empty((LD, B), dtype=np.float32)
    for i in range(NCORES):
        arr3 = np.asarray(res.results[i]["out_sh"]).astype(np.float32)
        arr2 = np.asarray(res.results[i]["out_sh2"]).astype(np.float32)
        j, h = i // 2, i % 2
        for p in range(3):
            l, dg = p // 2, p % 2
            r0 = 128 * (l * 8 + dg * 4 + j)
            xhatT[r0 : r0 + 128, h * NB : (h + 1) * NB] = arr3[p]
        # sub-blocks of (l=1, dg=1): core i holds tile t=i//2 rows (i%2)*64..
        jl, h2, rh = (i // 2) // 2, (i // 2) % 2, i % 2
        for sp in range(2):
            r0 = 128 * (12 + sp * 2 + jl) + rh * 64
            xhatT[r0 : r0 + 64, h2 * NB : (h2 + 1) * NB] = arr2[sp]
    return np.ascontiguousarray(xhatT.T).reshape(B, L, D).astype(np.float32)
